# revision 29
# baseline (speedup 1.0000x reference)
"""Trainium2 Bass kernel for nn_BaselineDNN (embedding-bag pooling + 2-layer MLP).

reference:
    emb = table[x]                       # [B, L, EMB] gather
    rep = emb.sum(1) / lengths[:, None]  # mean-pool over full L
    h = relu(rep @ W1 + b1)
    out = h @ W2 + b2

Data-parallel over batch across 8 NeuronCores (256 samples/core = 2 windows of
128). The cost of this kernel is dominated by the embedding gather's DMA
descriptor count (one 600B fp16 row per descriptor), so the host DEDUPLICATES
vocab rows per core: the core's 51200 tokens hit only ~40k distinct rows; each
distinct row is gathered ONCE and then accumulated into every (window, sample)
that references it via selection matmuls (sel[t,m] = 1 iff slot t's p-th
reference in window w is sample m; multi-reference slots get extra sel
passes). Slots are bucketed by vocab chunk (4 chunks of <=32768 rows so
indices fit int16), ordered [w0-only | both | w1-only] and by reference count
so multi-pass columns cluster; the compiled program is a data-dependent
envelope (max over the 8 cores) so one SPMD program serves all cores, with
surplus slots/passes on a given core masked by sid=-1 (sel all-zero).

sel matrices are built 16 passes per DVE op in an m-major layout
(sel[t, m*16+j] for pass j): every operand's innermost AP dim is then
stride-1 2-byte, which qualifies for the DVE 2x perf mode (the naive
[pass, m] layout broadcasts m innermost and runs at 1 elem/cycle). The
matmul reads pass j as a stride-16 lhsT column view. Gather indices are
DMA'd in the 16-partition wrapped layout only (1/8 the bytes) and
replicated to all 128 partitions with three doubling DVE copies on-chip.

The gather element is 600B (300 fp16) on a 768B row stride: the DMAGatherAnt
ISA only requires the STRIDE to be a multiple of 256B; bass's elem_size%256
assert is bypassed with a hand-built instruction (HW-verified exact). Pad
slots carry idx=0 (a real transfer) so every gather buffer is fully written
(masked junk*0 must not be NaN). Per-gather num_idxs is the exact padded
count, trimmed of trailing all-pad columns.

Lengths divide via reciprocal+multiply; the MLP runs on-chip (PE transposes +
matmuls; biases added via K=1 matmuls of a ones row), emitted per window as
soon as that window's accumulation completes so it overlaps remaining
gathers. Table cast to fp16 (error ~2e-4 rel; pooling accumulates in f32
PSUM).
"""

import numpy as np

import concourse.bacc as bacc
import concourse.mybir as mybir
import concourse.tile as tile
from concourse._compat import exact_div
from concourse.bass_utils import run_bass_kernel_spmd
from concourse.library_config import mlp as _mlp_lib

# Problem shapes (hardcoded per contract)
B, L, V, EMB, H, OUT = 2048, 200, 100000, 300, 128, 20
NCORES = 8
BC = B // NCORES          # samples per core (256)
P = 128
NW = BC // P              # windows per core (2)

GDT_NP = np.float16
GDT = mybir.dt.float16
SDT = mybir.dt.float16
SDT_NP = np.float16
DPAD = 384                # table row stride in elements (768 B, mult of 256)

CHUNK_BITS = 15
CHUNK_SZ = 1 << CHUNK_BITS           # 32768
NCHUNK = 4                           # ceil(100000 / 32768)
GN = 2048                            # max idxs per dma_gather instruction
GCOLS = GN // P                      # columns per full gather (16)
SELB = 16                            # sel passes built per DVE op
GBUFS = 13
SELBUFS = 12
IDX_STAGE_A = 4                      # gathers covered by the first idx stage
# chunk 3 (tiny) first: window 0's last matmul then lands ~93% through the
# stream, so its MLP overlaps the remaining gathers instead of the tail
CHUNK_ORDER = [3, 0, 1, 2]

F32 = mybir.dt.float32
I32 = mybir.dt.int32

_NC_CACHE = {}


def _manual_dma_gather(nc, out_ap, in_ap, idxs_ap, num_idxs, num_idxs_reg,
                       elem_size, elem_step):
    """bass.dma_gather without the elem_size%256 assert: the ISA only
    requires the row STRIDE to be a multiple of 256 bytes (stride_bytes_256
    field); the element byte count itself is free (HW-verified). Saves the
    row-padding bytes on every transfer."""
    g = nc.gpsimd
    stride_bytes = elem_step * mybir.dt.size(in_ap.dtype)
    stride_bytes_256 = exact_div(stride_bytes, 256)
    _in_ap = g.lower_ap_dma(in_ap, for_custom_bir_dma=True)
    _idxs_ap = g.lower_ap(idxs_ap)
    _out_ap = g.lower_ap(out_ap)
    return g.add_instruction(
        mybir.InstDMAGatherAnt(
            name=nc.get_next_instruction_name(),
            ins=[*_in_ap, _idxs_ap, g.lower_val_access(g.to_reg(num_idxs_reg))],
            outs=[_out_ap],
            transpose=False,
            num_idxs=num_idxs,
            elem_size=elem_size,
            stride_bytes_256=stride_bytes_256,
            gen_mode=0,
            single_packet=False,
            queue_num=0,
            sbuf_tokens_per_rank=0,
            sbuf_free_dim_per_rank=0,
            sbuf_free_dim_pad_per_rank=0,
            sbuf_byte_offset=0,
        )
    )


def _core_slots(x_core):
    """Dedup one core's tokens into per-chunk ordered slot lists.

    Returns per chunk: (local_idx [n] int, n0 [n], n1 [n]) ordered
    [w0-only | both | w1-only], refcount-desc within each region, plus
    per-token instance arrays (chunk, rank_in_chunk, w, sample, occ) where
    occ is the instance's occurrence rank within its (row, window) group.
    """
    v = x_core.ravel().astype(np.int64)
    w_arr = np.repeat(np.arange(NW, dtype=np.int64), P * L)
    s_arr = np.tile(np.repeat(np.arange(P, dtype=np.int64), L), NW)

    uniq, inv = np.unique(v, return_inverse=True)
    nu = len(uniq)
    n_w = np.zeros((nu, NW), dtype=np.int64)
    np.add.at(n_w, (inv, w_arr), 1)
    n0, n1 = n_w[:, 0], n_w[:, 1]

    # occurrence rank of each token within its (row, window) group
    order = np.lexsort((s_arr, w_arr, inv))
    key = inv[order] * NW + w_arr[order]
    first = np.r_[True, key[1:] != key[:-1]]
    grp_start = np.maximum.accumulate(np.where(first, np.arange(len(key)), 0))
    occ_sorted = np.arange(len(key)) - grp_start
    occ = np.empty(len(key), dtype=np.int64)
    occ[order] = occ_sorted

    # order unique rows: chunk-major, [w0-only | both | w1-only], count desc
    chunk = uniq >> CHUNK_BITS
    cat = np.where(n1 == 0, 0, np.where(n0 > 0, 1, 2))
    tot = n0 + n1
    perm = np.lexsort((-tot, cat, chunk))   # final sort key order: chunk,cat,-tot
    # rank of each unique row within its chunk after ordering
    rank = np.empty(nu, dtype=np.int64)
    chunk_sorted = chunk[perm]
    cstart = np.searchsorted(chunk_sorted, np.arange(NCHUNK))
    pos = np.arange(nu)
    rank[perm] = pos - cstart[chunk_sorted]

    per_chunk = []
    for k in range(NCHUNK):
        sel = chunk_sorted == k
        idxs = uniq[perm[sel]] & (CHUNK_SZ - 1)
        per_chunk.append((idxs.astype(np.int16),
                          n0[perm[sel]], n1[perm[sel]]))

    inst = dict(chunk=chunk[inv], rank=rank[inv], w=w_arr, s=s_arr, occ=occ)
    return per_chunk, inst


def _make_schedule(cores):
    """Envelope schedule across cores (identical program structure).

    ncols[k]       columns per chunk
    col_base[k]    global column index of chunk k's first column
    npass[gcol][w] sel passes per global column and window
    gathers        list of (chunk, col_lo, col_hi, col_hi_eff): trailing
                   columns with zero passes on every core are not gathered
    sid_base[gcol][w] first sid column of (gcol, w) passes
    S / S_pad      total sid columns (and padded to a multiple of SELB)
    mm_total[w]    total matmuls per window
    """
    ncols = []
    for k in range(NCHUNK):
        n = max(len(pc[k][0]) for pc, _ in cores)
        ncols.append(max(1, -(-n // P)))
    col_base = np.zeros(NCHUNK + 1, dtype=np.int64)
    run = 0
    for k in CHUNK_ORDER:
        col_base[k] = run
        run += ncols[k]
    col_base[NCHUNK] = run          # sentinel: total columns
    tot_cols = int(run)

    npass = np.zeros((tot_cols, NW), dtype=np.int64)
    for pc, _ in cores:
        for k in range(NCHUNK):
            idxs, n0, n1 = pc[k]
            if len(idxs) == 0:
                continue
            cols = col_base[k] + np.arange(len(idxs)) // P
            np.maximum.at(npass[:, 0], cols, n0)
            np.maximum.at(npass[:, 1], cols, n1)

    # stripe merge-heavy columns evenly within each (chunk, region-class):
    # the refcount-desc slot packing clusters multi-pass columns at region
    # heads, which bursts the DVE (sel merges) and starves PE; spacing them
    # keeps per-batch DVE load even. Permuting whole columns preserves the
    # envelope (slots move with their column) and the region order.
    colperm = np.arange(tot_cols)
    for k in range(NCHUNK):
        cb = int(col_base[k])
        cols = np.arange(cb, cb + ncols[k])
        w0p = npass[cols, 0] > 0
        w1p = npass[cols, 1] > 0
        cls = np.where(w0p & w1p, 1, np.where(w0p, 0, np.where(w1p, 2, 3)))
        new_order = []
        for rc in (0, 1, 2, 3):
            sub = [int(c) for c in cols[cls == rc]]
            heavy = [c for c in sub if int(npass[c].sum()) >= 3]
            light = [c for c in sub if int(npass[c].sum()) < 3]
            if heavy and light:
                n, h = len(sub), len(heavy)
                hpos = set(int(np.floor((i + 0.5) * n / h)) for i in range(h))
                merged, hi, li = [], 0, 0
                for p_ in range(n):
                    if p_ in hpos and hi < h:
                        merged.append(heavy[hi]); hi += 1
                    elif li < len(light):
                        merged.append(light[li]); li += 1
                    else:
                        merged.append(heavy[hi]); hi += 1
                sub = merged
            new_order.extend(sub)
        for newpos, oc in enumerate(new_order):
            colperm[oc] = cb + newpos
    npass2 = np.zeros_like(npass)
    npass2[colperm] = npass
    npass = npass2

    gathers = []
    for k in CHUNK_ORDER:
        c0 = int(col_base[k])
        cend = c0 + ncols[k]
        while c0 < cend:
            c1 = min(c0 + GCOLS, cend)
            used = [c for c in range(c0, c1) if npass[c].sum() > 0]
            c1_eff = (max(used) + 1) if used else c0
            gathers.append((k, c0, c1, c1_eff))
            c0 = c1

    # sid column assignment. Columns with k>=3 passes get 2 matmuls: one raw
    # pass + one DVE-merged (tensor_reduce) matrix over passes 2..k; the
    # merge slice must sit inside a single SELB-wide sel tile, so such
    # groups are aligned to not straddle a batch boundary.
    sid_base = np.zeros((tot_cols, NW), dtype=np.int64)
    groups = []   # (gcol, w, sid_start, k)
    s = 0
    for (k_, c0, c1, _) in gathers:
        for c in range(c0, c1):
            for w in range(NW):
                k = int(npass[c, w])
                if k == 0:
                    continue
                if k >= 3 and (s // SELB) != ((s + k - 1) // SELB):
                    s = (s // SELB + 1) * SELB
                sid_base[c, w] = s
                groups.append((c, w, s, k))
                s += k
    S_pad = -(-max(s, 1) // SELB) * SELB
    mm_total = [sum(min(k, 2) for _, w_, _, k in groups if w_ == w)
                for w in range(NW)]
    return dict(ncols=ncols, col_base=col_base, tot_cols=tot_cols,
                npass=npass, gathers=gathers, sid_base=sid_base, S=s,
                S_pad=S_pad, mm_total=mm_total, groups=groups,
                colperm=colperm)


def _schedule_key(sched):
    return (tuple(sched["ncols"]), sched["S"],
            tuple(map(tuple, sched["npass"].tolist())),
            tuple(sched["gathers"]))


def _fill_core(sched, pc, inst):
    """Build one core's idx16 tile [16, TOT//16] and sid tile [128, S_pad]."""
    col_base = sched["col_base"]
    colperm = sched["colperm"]
    tot_slots = sched["tot_cols"] * P
    idx_stream = np.zeros(tot_slots, dtype=np.int16)   # pad slots gather row 0
    sid_flat = np.full(sched["S_pad"] * P, -1.0, dtype=SDT_NP)

    for k in range(NCHUNK):
        idxs, _, _ = pc[k]
        r = np.arange(len(idxs))
        pos = colperm[col_base[k] + r // P] * P + r % P
        idx_stream[pos] = idxs

    # token instances -> sid columns
    gcol = colperm[col_base[inst["chunk"]] + inst["rank"] // P]
    t = inst["rank"] % P
    sidcol = sched["sid_base"][gcol, inst["w"]] + inst["occ"]
    sid_flat[sidcol * P + t] = inst["s"].astype(SDT_NP)

    idx32 = np.tile(idx_stream.reshape(tot_slots // 16, 16).T, (2, 1))
    sid_tile = sid_flat.reshape(sched["S_pad"], P).T.copy()
    return idx32, sid_tile


def _build_nc(sched):
    nc = bacc.Bacc(
        "TRN2", target_bir_lowering=False, debug=False, enable_asserts=False
    )
    tot_cols = sched["tot_cols"]
    S_pad = sched["S_pad"]
    npass = sched["npass"]
    gathers = sched["gathers"]
    TOT = tot_cols * P

    idx_d = nc.dram_tensor("idx", [32, TOT // 16], mybir.dt.int16, kind="ExternalInput")
    sid_d = nc.dram_tensor("sid", [P, S_pad], SDT, kind="ExternalInput")
    miota_d = nc.dram_tensor("miota", [P, P], SDT, kind="ExternalInput")
    len_d = nc.dram_tensor("lens", [BC, 1], I32, kind="ExternalInput")
    tab_d = nc.dram_tensor("table", [V, DPAD], GDT, kind="ExternalInput")
    w1_d = nc.dram_tensor("W1", [EMB, H], F32, kind="ExternalInput")
    b1_d = nc.dram_tensor("b1", [1, H], F32, kind="ExternalInput")
    w2_d = nc.dram_tensor("W2", [H, OUT], F32, kind="ExternalInput")
    b2_d = nc.dram_tensor("b2", [1, OUT], F32, kind="ExternalInput")
    out_d = nc.dram_tensor("out", [BC, OUT], F32, kind="ExternalOutput")

    emb_chunks = [(0, 128), (128, 128), (256, EMB - 256)]

    # column -> gather index / column-within-gather
    col2g = {}
    for gi, (k, c0, c1, c1e) in enumerate(gathers):
        for c in range(c0, c1):
            col2g[c] = (gi, c - c0)

    # actions in sid order: ("raw", sidcol, gi, cg, w) one matmul of one
    # pass; ("merge", sidcol, nmats, gi, cg, w) one tensor_reduce over
    # nmats pass matrices + one matmul.
    actions = []
    for (c, w, s0, k) in sched["groups"]:
        gi, cg = col2g[c]
        if k <= 2:
            for p_ in range(k):
                actions.append(("raw", s0 + p_, 1, gi, cg, w))
        else:
            actions.append(("raw", s0, 1, gi, cg, w))
            actions.append(("merge", s0 + 1, k - 1, gi, cg, w))
    n_batch = sched["S_pad"] // SELB
    batch_actions = [[] for _ in range(n_batch)]
    for a in actions:
        batch_actions[a[1] // SELB].append(a)

    with tile.TileContext(nc) as tc:
        with (
            tc.tile_pool(name="const", bufs=1) as cp,
            tc.tile_pool(name="g", bufs=GBUFS) as gp,
            tc.tile_pool(name="sel", bufs=SELBUFS) as selp,
            tc.tile_pool(name="mlp", bufs=2) as mp,
            tc.tile_pool(name="acc", bufs=1, space="PSUM") as accp,
            tc.tile_pool(name="psmall", bufs=1, space="PSUM") as psp,
            tc.tile_pool(name="ptr", bufs=2, space="PSUM") as ptrp,
        ):
            nc.gpsimd.load_library(_mlp_lib)

            # idx: DMA the 16-partition wrap twice (32 partitions), replicate
            # to 128 with two 32-aligned doubling DVE copies (engine partition
            # windows must start at multiples of 32); staged so gather 0
            # starts immediately.
            idx_t = cp.tile([P, TOT // 16], mybir.dt.int16)
            hw_ = TOT // 16
            cutA = min(IDX_STAGE_A * GN // 16, hw_)
            for lo, hi in ((0, cutA), (cutA, hw_)):
                if hi <= lo:
                    continue
                nc.sync.dma_start(out=idx_t[0:32, lo:hi],
                                  in_=idx_d.ap()[:, lo:hi])
                for pp in (32, 64):
                    nc.vector.tensor_copy(out=idx_t[pp:2 * pp, lo:hi],
                                          in_=idx_t[0:pp, lo:hi])

            # sel-build constants
            miota = cp.tile([P, P], SDT)
            nc.sync.dma_start(out=miota[:], in_=miota_d.ap())
            sid_t = cp.tile([P, S_pad], SDT)
            nc.sync.dma_start(out=sid_t[:], in_=sid_d.ap())
            mrep = cp.tile([P, P * SELB], SDT)
            nc.vector.tensor_copy(
                out=mrep[:].rearrange("p (m j) -> p m j", j=SELB),
                in_=miota[:].unsqueeze(2).to_broadcast([P, P, SELB]),
            )

            ident = cp.tile([P, P], F32)
            from concourse.masks import make_identity

            make_identity(nc, ident[:])
            ones1 = cp.tile([1, P], F32)
            nc.vector.memset(ones1[:], 1.0)

            len_t = cp.tile([P, NW], I32)
            nc.sync.dma_start(
                out=len_t[:], in_=len_d.ap().rearrange("(w p) o -> p (w o)", p=P)
            )
            len_f = cp.tile([P, NW], F32)
            nc.vector.tensor_copy(out=len_f[:], in_=len_t[:])
            inv_len = cp.tile([P, NW], F32)
            nc.vector.reciprocal(out=inv_len[:], in_=len_f[:])

            # MLP weights (needed only ~60us in; loaded after gathers start)
            w1s = []
            for e, (off, wd) in enumerate(emb_chunks):
                t = cp.tile([P, H], F32, tag=f"w1_{e}", name=f"w1_{e}")
                w1s.append(t)
            b1t = cp.tile([1, H], F32)
            w2t = cp.tile([P, OUT], F32)
            b2t = cp.tile([1, OUT], F32)

            def _load_weights():
                for e, (off, wd) in enumerate(emb_chunks):
                    nc.sync.dma_start(out=w1s[e][:wd, :],
                                      in_=w1_d.ap()[off:off + wd, :])
                nc.sync.dma_start(out=b1t[:], in_=b1_d.ap())
                nc.sync.dma_start(out=w2t[:], in_=w2_d.ap())
                nc.sync.dma_start(out=b2t[:], in_=b2_d.ap())

            accs = [accp.tile([P, EMB], F32, tag=f"acc{w}", name=f"acc{w}",
                              space="PSUM")
                    for w in range(NW)]
            mm_done = [0] * NW
            mm_total = sched["mm_total"]
            mlp_emitted = [False] * NW

            def _mlp_stages(w):
                """MLP as a list of small closures, each anchored on 1-2 PE
                ops; interleaving them with pooling matmuls hides the
                cross-engine (DVE copy / Act relu) latencies that would
                otherwise stall PE's in-order queue."""
                st = {}

                def s_div():
                    st["rep"] = mp.tile([P, EMB], F32, tag=f"rep{w}",
                                        name="rep")
                    nc.vector.tensor_scalar(
                        out=st["rep"][:], in0=accs[w][:],
                        scalar1=inv_len[:, w:w + 1], scalar2=None,
                        op0=mybir.AluOpType.mult,
                    )
                    st["h_ps"] = psp.tile([P, H], F32, tag="h_ps",
                                          name="h_ps", space="PSUM")

                def s_tr(e):
                    def f():
                        off, wd = emb_chunks[e]
                        rt_ps = ptrp.tile([P, P], F32, tag="rt_ps",
                                          name="rt_ps", space="PSUM")
                        nc.tensor.transpose(out=rt_ps[:wd, :],
                                            in_=st["rep"][:, off:off + wd],
                                            identity=ident[:])
                        rt = mp.tile([P, P], F32, tag=f"rt{e}", name="rt")
                        nc.vector.tensor_copy(out=rt[:wd, :],
                                              in_=rt_ps[:wd, :])
                        st[f"rt{e}"] = rt
                    return f

                def s_mm(e):
                    def f():
                        off, wd = emb_chunks[e]
                        nc.tensor.matmul(
                            out=st["h_ps"][:], lhsT=st[f"rt{e}"][:wd, :],
                            rhs=w1s[e][:wd, :], start=(e == 0), stop=False,
                        )
                        if e == len(emb_chunks) - 1:
                            nc.tensor.matmul(
                                out=st["h_ps"][:], lhsT=ones1[:], rhs=b1t[:],
                                start=False, stop=True,
                            )
                            h = mp.tile([P, H], F32, tag=f"h{w}", name="h")
                            nc.scalar.activation(
                                out=h[:], in_=st["h_ps"][:],
                                func=mybir.ActivationFunctionType.Relu,
                            )
                            st["h"] = h
                    return f

                def s_ht():
                    ht_ps = psp.tile([P, P], F32, tag="ht_ps", name="ht_ps", space="PSUM")
                    nc.tensor.transpose(out=ht_ps[:], in_=st["h"][:],
                                        identity=ident[:])
                    ht = mp.tile([P, P], F32, tag=f"ht{w}", name="ht")
                    nc.vector.tensor_copy(out=ht[:], in_=ht_ps[:])
                    st["ht"] = ht

                def s_out():
                    o_ps = psp.tile([P, OUT], F32, tag="o_ps", name="o_ps", space="PSUM")
                    nc.tensor.matmul(out=o_ps[:], lhsT=st["ht"][:],
                                     rhs=w2t[:], start=True, stop=False)
                    nc.tensor.matmul(out=o_ps[:], lhsT=ones1[:], rhs=b2t[:],
                                     start=False, stop=True)
                    o_t = mp.tile([P, OUT], F32, tag=f"o_t{w}", name="o_t")
                    nc.vector.tensor_copy(out=o_t[:], in_=o_ps[:])
                    nc.sync.dma_start(out=out_d.ap()[w * P:(w + 1) * P, :],
                                      in_=o_t[:])

                return [s_div, s_tr(0), s_tr(1), s_tr(2), s_mm(0), s_mm(1),
                        s_mm(2), s_ht, s_out]

            pending_stages = []

            def _emit_mlp(w):
                pending_stages.extend(_mlp_stages(w))

            gtiles = {}
            emitted_g = 0
            act_count = 0

            def _emit_gather(gi):
                k, c0, c1, c1e = gathers[gi]
                ncols_g = c1e - c0
                if ncols_g <= 0:
                    gtiles[gi] = None
                    return
                gn = ncols_g * P
                base_row = k * CHUNK_SZ
                rows = min(CHUNK_SZ, V - base_row)
                slot_base = c0 * P
                g = gp.tile([P, GCOLS * EMB], GDT, tag="g", name="g")
                gv = g[:, : ncols_g * EMB].rearrange("p (s e) -> p s e", s=ncols_g)
                _manual_dma_gather(
                    nc,
                    gv,
                    tab_d.ap()[base_row : base_row + rows, :EMB],
                    idx_t[:, slot_base // 16 : (slot_base + gn) // 16],
                    gn,
                    gn,
                    EMB,
                    DPAD,
                )
                gtiles[gi] = gv

            for bi in range(n_batch):
                acts = batch_actions[bi]
                if not acts:
                    continue
                need_g = max(a[3] for a in acts)
                while emitted_g <= need_g:
                    _emit_gather(emitted_g)
                    emitted_g += 1
                    if emitted_g == 2:
                        _load_weights()
                b0 = bi * SELB
                sel = selp.tile([P, P * SELB], SDT, tag="sel", name="sel")
                selmj = sel[:].rearrange("p (m j) -> p m j", j=SELB)
                nc.vector.tensor_tensor(
                    out=selmj,
                    in0=sid_t[:, b0:b0 + SELB]
                    .unsqueeze(1)
                    .to_broadcast([P, P, SELB]),
                    in1=mrep[:].rearrange("p (m j) -> p m j", j=SELB),
                    op=mybir.AluOpType.is_equal,
                )
                selv = sel[:].rearrange("p (m j) -> p j m", j=SELB)
                for (kind, scol, nmats, gi, cg, w) in acts:
                    jj = scol - b0
                    if kind == "raw":
                        lhsT = selv[:, jj, :]
                    else:
                        merged = selp.tile([P, P], SDT, tag="selm",
                                           name="selm")
                        with nc.allow_low_precision(
                            reason="0/1 sel counts <=16 exact in f16"
                        ):
                            nc.vector.tensor_reduce(
                                out=merged[:],
                                in_=selmj[:, :, jj:jj + nmats],
                                axis=mybir.AxisListType.X,
                                op=mybir.AluOpType.add,
                            )
                        lhsT = merged[:]
                    nc.tensor.matmul(
                        out=accs[w][:],
                        lhsT=lhsT,
                        rhs=gtiles[gi][:, cg, :],
                        start=(mm_done[w] == 0),
                        stop=(mm_done[w] == mm_total[w] - 1),
                    )
                    mm_done[w] += 1
                    act_count += 1
                    if pending_stages and act_count % 8 == 0:
                        pending_stages.pop(0)()
                for w in range(NW):
                    if not mlp_emitted[w] and mm_done[w] == mm_total[w]:
                        _emit_mlp(w)
                        mlp_emitted[w] = True

            for w in range(NW):
                if not mlp_emitted[w]:
                    _emit_mlp(w)
                    mlp_emitted[w] = True
            while pending_stages:
                pending_stages.pop(0)()

    nc.compile()
    return nc


def get_nc():
    if "nc" not in _NC_CACHE:
        raise RuntimeError("kernel() must run once before get_nc()")
    return _NC_CACHE["nc"]


def make_in_maps(x, lengths, emb_table, W1, b1, W2, b2):
    x = np.ascontiguousarray(x).astype(np.int64, copy=False)
    lengths = np.ascontiguousarray(lengths.astype(np.int32, copy=False)).reshape(B, 1)
    tab = np.zeros((V, DPAD), dtype=GDT_NP)
    tab[:, :EMB] = emb_table.astype(GDT_NP, copy=False)
    W1 = np.ascontiguousarray(W1.astype(np.float32, copy=False))
    b1 = np.ascontiguousarray(b1.astype(np.float32, copy=False)).reshape(1, H)
    W2 = np.ascontiguousarray(W2.astype(np.float32, copy=False))
    b2 = np.ascontiguousarray(b2.astype(np.float32, copy=False)).reshape(1, OUT)
    miota = np.tile(np.arange(P, dtype=SDT_NP), (P, 1))

    cores = [_core_slots(x[c * BC : (c + 1) * BC]) for c in range(NCORES)]
    sched = _make_schedule(cores)

    in_maps = []
    for c in range(NCORES):
        pc, inst = cores[c]
        idx16, sid_tile = _fill_core(sched, pc, inst)
        in_maps.append(
            {
                "idx": idx16,
                "sid": sid_tile,
                "miota": miota,
                "lens": lengths[c * BC : (c + 1) * BC],
                "table": tab,
                "W1": W1,
                "b1": b1,
                "W2": W2,
                "b2": b2,
            }
        )
    return sched, in_maps


def kernel(x, lengths, emb_table, W1, b1, W2, b2):
    sched, in_maps = make_in_maps(x, lengths, emb_table, W1, b1, W2, b2)
    key = _schedule_key(sched)
    if _NC_CACHE.get("key") != key:
        _NC_CACHE["nc"] = _build_nc(sched)
        _NC_CACHE["key"] = key
    nc = _NC_CACHE["nc"]
    res = run_bass_kernel_spmd(nc, in_maps, core_ids=list(range(NCORES)))
    return np.concatenate([r["out"] for r in res.results], axis=0)
